# revision 4
# baseline (speedup 1.0000x reference)
"""GCN message-passing kernel (relu(GCNConv(x, edge_index)) w/ symmetric norm)
for 8 trn2 NeuronCores.

Math: out = relu( D^-1/2 (A+I) D^-1/2 (x @ W) + b )
Rewritten:  out[d] = relu( dinv[d] * (sum_{e: dst=d} xs[src_e]) @ W + b )
where xs[i] = dinv[i] * x[i]  (host-precomputed fp16 gather table).

Device work per core (12500 dst nodes, ~212k edges):
  - dma_gather 256B fp16 xs rows per edge (dst-sorted, binned into nbins bins
    of <=128 dst slots; src space split into 4 blocks of 25000 rows so int16
    gather indices fit; per-core block remap puts the self-loop-heavy block
    first, which gets a bigger chunk budget)
  - one-hot matmul segment sum: psum[k, slot] += xg[e,k]^T @ S[e, slot]
    (S built on DVE via iota==slot compare)
  - per bin: scale cols by dinv[dst], @W matmul (+bias via ones@brep), relu
Host: index prep / sharding / unpermute only (plus fp16 cast of x*dinv).
"""

import numpy as np

import concourse.bacc as bacc
import concourse.mybir as mybir
import concourse.tile as tile
from concourse.bass_utils import run_bass_kernel_spmd

F16 = mybir.dt.float16
F32 = mybir.dt.float32
I16 = mybir.dt.int16


class Cfg:
    def __init__(self, n_nodes, n_cores, nblk, nbins, bpg, cpbs):
        self.n_nodes = n_nodes
        self.n_cores = n_cores
        self.shard = n_nodes // n_cores      # dst nodes per core
        self.nblk = nblk                     # src blocks (int16 index range)
        self.blk = n_nodes // nblk           # rows per src block (< 32768)
        self.nbins = nbins                   # bins per core (128 dst slots each)
        self.bpg = bpg                       # bins per gather super-group
        self.nsg = nbins // bpg              # super-groups
        self.cpbs = tuple(cpbs)              # per-block chunks (128 edges) per bin
        self.caps = tuple(c * 128 for c in cpbs)
        assert len(cpbs) == nblk
        assert n_nodes % n_cores == 0 and n_nodes % nblk == 0
        assert self.blk < 32768 and nbins % bpg == 0
        assert self.shard <= nbins * 128
        # per-core block remap assumes a shard is contained in one block
        assert self.blk % self.shard == 0 or self.shard % self.blk == 0
        assert self.blk >= self.shard
        self.d_in = 128
        self.d_out = 64
        self.ncol_sg = bpg * sum(cpbs)       # slot cols per super-group
        self.boff = tuple(bpg * int(np.sum(cpbs[:b])) for b in range(nblk))

    def key(self):
        return (self.n_nodes, self.n_cores, self.nblk, self.nbins, self.bpg,
                self.cpbs)


FULL = Cfg(n_nodes=100000, n_cores=8, nblk=4, nbins=112, bpg=7,
           cpbs=(5, 4, 4, 4))


# ----------------------------------------------------------------------------
# host-side prep: shard / bin / build index+slot streams
# ----------------------------------------------------------------------------

def _pack_bins(cfg, degv):
    """Assign each dst (of one core's shard) to a bin s.t. per-(bin, blk) edge
    counts fit cfg.caps[blk] and bins hold <=128 dsts. Greedy fill by
    normalized worst-block load, then swap repair. Returns bin_of [shard]."""
    shard, nbins = cfg.shard, cfg.nbins
    caps = np.array(cfg.caps, np.float64)
    order = np.argsort(-degv.sum(1), kind="stable")
    loads = np.zeros((nbins, cfg.nblk), np.float64)
    counts = np.zeros(nbins, np.int64)
    bin_of = np.empty(shard, np.int64)
    for d in order:
        score = ((loads + degv[d]) / caps).max(1)
        score[counts >= 128] = np.inf
        b = int(np.argmin(score))
        bin_of[d] = b
        loads[b] += degv[d]
        counts[b] += 1

    loads = loads.astype(np.int64)
    capsi = np.array(cfg.caps, np.int64)
    for _ in range(2000):
        over = loads - capsi[None, :]
        wb, wk = np.unravel_index(np.argmax(over), over.shape)
        if over[wb, wk] <= 0:
            break
        done = False
        in_wb = np.where(bin_of == wb)[0]
        in_wb = in_wb[np.argsort(-degv[in_wb, wk])][:16]
        for tb in np.argsort(loads[:, wk])[:24]:
            if tb == wb:
                continue
            in_tb = np.where(bin_of == tb)[0]
            in_tb = in_tb[np.argsort(degv[in_tb, wk])][:16]
            for d in in_wb:
                for e in in_tb:
                    na = loads[wb] - degv[d] + degv[e]
                    nb = loads[tb] - degv[e] + degv[d]
                    if (na <= capsi).all() and (nb <= capsi).all() \
                            and na[wk] < loads[wb, wk]:
                        loads[wb], loads[tb] = na, nb
                        bin_of[d], bin_of[e] = tb, wb
                        done = True
                        break
                if done:
                    break
            if done:
                break
        if not done:
            raise RuntimeError(f"bin packing failed (load {loads.max(0)}, "
                               f"caps {cfg.caps}); raise cpbs")
    assert (loads <= capsi[None, :]).all()
    assert np.bincount(bin_of, minlength=nbins).max() <= 128
    return bin_of


def prep(cfg, x, edge_index, weight, bias):
    """Returns (in_maps, unperms). in_maps: per-core dict of named np arrays.
    unperms[m][d] = row in core m's output holding dst (m*shard + d)."""
    n, shard, nblk, blk = cfg.n_nodes, cfg.shard, cfg.nblk, cfg.blk
    nbins, bpg, nsg, cpbs, caps = cfg.nbins, cfg.bpg, cfg.nsg, cfg.cpbs, cfg.caps

    src = np.asarray(edge_index[0], dtype=np.int64)
    dst = np.asarray(edge_index[1], dtype=np.int64)
    loop = np.arange(n, dtype=np.int64)
    src_f = np.concatenate([src, loop])
    dst_f = np.concatenate([dst, loop])

    deg = np.bincount(dst_f, minlength=n).astype(np.float32)
    dinv = np.where(deg > 0, 1.0 / np.sqrt(deg), 0.0).astype(np.float32)

    xs16 = (np.asarray(x, np.float32) * dinv[:, None]).astype(np.float16)
    xb = []
    for b in range(nblk):
        t = np.zeros((blk + 1, cfg.d_in), np.float16)
        t[:blk] = xs16[b * blk:(b + 1) * blk]
        xb.append(t)

    w32 = np.asarray(weight, np.float32)                       # [128, 64]
    b32 = np.asarray(bias, np.float32)
    ones = np.ones((128, 128), np.float32)
    brep = np.tile((b32 / 128.0)[None, :], (128, 1)).astype(np.float32)
    iota = np.tile(np.arange(128, dtype=np.float16)[None, :], (128, 1))

    # sort edges by dst once, globally
    order_all = np.argsort(dst_f, kind="stable")
    src_s = src_f[order_all]
    dst_s = dst_f[order_all]
    bounds = np.searchsorted(dst_s, np.arange(0, n + 1, shard))

    in_maps, unperms = [], []
    for m in range(cfg.n_cores):
        lo, hi = bounds[m], bounds[m + 1]
        e_src = src_s[lo:hi]
        e_dst = dst_s[lo:hi] - m * shard
        # per-core block remap: self-loop block (containing this shard's own
        # rows) becomes logical block 0 (which has the larger chunk budget)
        selfblk = (m * shard) // blk
        perm = [selfblk] + [b for b in range(nblk) if b != selfblk]
        inv = np.empty(nblk, np.int64)
        for lb, pb in enumerate(perm):
            inv[pb] = lb
        e_blk = inv[e_src // blk]
        degv = np.bincount(e_dst * nblk + e_blk, minlength=shard * nblk) \
                 .reshape(shard, nblk)
        bin_of = _pack_bins(cfg, degv)

        # slots: rank of dst within its bin
        counts = np.bincount(bin_of, minlength=nbins)
        starts = np.concatenate([[0], np.cumsum(counts)[:-1]])
        by_bin = np.argsort(bin_of, kind="stable")
        slot_of = np.empty(shard, np.int64)
        slot_of[by_bin] = np.arange(shard) - np.repeat(starts, counts)

        # order edges by (bin, blk, slot); position within (bin, blk) group
        b_e = bin_of[e_dst]
        s_e = slot_of[e_dst]
        es = np.lexsort((s_e, e_blk, b_e))
        gids = b_e[es] * nblk + e_blk[es]
        gcnt = np.bincount(gids, minlength=nbins * nblk)
        gstart = np.concatenate([[0], np.cumsum(gcnt)[:-1]])
        within = np.arange(len(es)) - np.repeat(gstart, gcnt)

        eb = e_blk[es]
        locs = (e_src[es] % blk).astype(np.int16)
        sl = s_e[es].astype(np.float32)
        binpos = b_e[es]

        im = {}
        slots_t = np.zeros((128, nsg * cfg.ncol_sg), np.float32)
        for b in range(nblk):
            cap = caps[b]
            call = bpg * cap
            idx_stream = np.full(nbins * cap, blk, np.int16)
            slot_stream = np.zeros(nbins * cap, np.float32)
            msk = eb == b
            p = binpos[msk] * cap + within[msk]
            idx_stream[p] = locs[msk]
            slot_stream[p] = sl[msk]

            segs = []
            for sg in range(nsg):
                seg = idx_stream[sg * call:(sg + 1) * call]
                for o in range(0, call, 1024):
                    segs.append(seg[o:o + 1024].reshape(-1, 16).T)
            arr16 = np.concatenate(segs, axis=1)
            im[f"idx{b}"] = np.tile(arr16, (8, 1)).copy()
            im[f"xb{b}"] = xb[perm[b]]

            ss = slot_stream.reshape(nbins, cpbs[b], 128)
            for sg in range(nsg):
                cols = ss[sg * bpg:(sg + 1) * bpg].reshape(bpg * cpbs[b], 128).T
                c0 = sg * cfg.ncol_sg + cfg.boff[b]
                slots_t[:, c0:c0 + bpg * cpbs[b]] = cols
        im["slots"] = slots_t

        drow = np.zeros(nbins * 128, np.float32)
        drow[bin_of * 128 + slot_of] = dinv[m * shard + np.arange(shard)]
        im["drep"] = np.tile(drow[None, :], (128, 1)).copy()

        im["w"] = w32
        im["ones"] = ones
        im["brep"] = brep
        im["iota"] = iota
        in_maps.append(im)
        unperms.append(bin_of * 128 + slot_of)
    return in_maps, unperms


# ----------------------------------------------------------------------------
# device kernel
# ----------------------------------------------------------------------------

def build_nc(cfg):
    nblk, nbins, bpg, nsg, cpbs = cfg.nblk, cfg.nbins, cfg.bpg, cfg.nsg, cfg.cpbs
    ncol_sg = cfg.ncol_sg
    nc = bacc.Bacc("TRN2", target_bir_lowering=False, debug=False,
                   num_devices=cfg.n_cores)

    xb = [nc.dram_tensor(f"xb{b}", [cfg.blk + 1, cfg.d_in], F16,
                         kind="ExternalInput") for b in range(nblk)]
    idxt = [nc.dram_tensor(f"idx{b}", [128, nsg * bpg * cfg.caps[b] // 16], I16,
                           kind="ExternalInput") for b in range(nblk)]
    slott = nc.dram_tensor("slots", [128, nsg * ncol_sg], F32,
                           kind="ExternalInput")
    drept = nc.dram_tensor("drep", [128, nbins * 128], F32, kind="ExternalInput")
    wt = nc.dram_tensor("w", [cfg.d_in, cfg.d_out], F32, kind="ExternalInput")
    onest = nc.dram_tensor("ones", [128, 128], F32, kind="ExternalInput")
    brept = nc.dram_tensor("brep", [128, cfg.d_out], F32, kind="ExternalInput")
    iotat = nc.dram_tensor("iota", [128, 128], F16, kind="ExternalInput")
    outt = nc.dram_tensor("out", [nbins * 128, cfg.d_out], F32,
                          kind="ExternalOutput")

    with tile.TileContext(nc) as tc:
        with tc.tile_pool(name="const", bufs=1) as cpool, \
             tc.tile_pool(name="work", bufs=1) as wpool, \
             tc.tile_pool(name="psumT", bufs=3, space="PSUM") as ppool, \
             tc.tile_pool(name="psum2", bufs=2, space="PSUM") as p2pool:

            iota_s = cpool.tile([128, 128], F16, name="iota_s")
            nc.sync.dma_start(out=iota_s[:], in_=iotat[:])
            w_s = cpool.tile([cfg.d_in, cfg.d_out], F32, name="w_s")
            nc.sync.dma_start(out=w_s[:], in_=wt[:])
            ones_s = cpool.tile([128, 128], F32, name="ones_s")
            nc.sync.dma_start(out=ones_s[:], in_=onest[:])
            brep_s = cpool.tile([128, cfg.d_out], F32, name="brep_s")
            nc.sync.dma_start(out=brep_s[:], in_=brept[:])

            for sg in range(nsg):
                idx_tiles = []
                for b in range(nblk):
                    w16 = bpg * cfg.caps[b] // 16
                    it = wpool.tile([128, w16], I16, name=f"it{b}",
                                    tag=f"it{b}", bufs=2)
                    nc.sync.dma_start(
                        out=it[:], in_=idxt[b][:, sg * w16:(sg + 1) * w16])
                    idx_tiles.append(it)
                slot_s = wpool.tile([128, ncol_sg], F32, name="slot_s",
                                    tag="slot", bufs=2)
                nc.sync.dma_start(
                    out=slot_s[:],
                    in_=slott[:, sg * ncol_sg:(sg + 1) * ncol_sg])
                drep_s = wpool.tile([128, bpg * 128], F32, name="drep_s",
                                    tag="drep", bufs=2)
                nc.sync.dma_start(
                    out=drep_s[:],
                    in_=drept[:, sg * bpg * 128:(sg + 1) * bpg * 128])

                xg = []
                for b in range(nblk):
                    call = bpg * cfg.caps[b]
                    g = wpool.tile([128, bpg * cpbs[b], cfg.d_in], F16,
                                   name=f"xg{b}", tag=f"xg{b}", bufs=2)
                    for o in range(0, call, 1024):
                        nloc = min(1024, call - o)
                        nc.gpsimd.dma_gather(
                            g[:, o // 128:(o + nloc) // 128, :], xb[b][:],
                            idx_tiles[b][:, o // 16:(o + nloc) // 16],
                            nloc, nloc, cfg.d_in)
                    xg.append(g)

                outst = wpool.tile([128, bpg, cfg.d_out], F32, name="outst",
                                   tag="outst", bufs=2)
                for b7 in range(bpg):
                    pT = ppool.tile([128, 128], F32, name="pT")
                    k = 0
                    nmm = sum(cpbs)
                    for b in range(nblk):
                        for j in range(cpbs[b]):
                            s_t = wpool.tile([128, 128], F16, name="s_t",
                                             tag="s_t", bufs=4)
                            c = cfg.boff[b] + b7 * cpbs[b] + j
                            nc.vector.tensor_scalar(
                                s_t[:], iota_s[:], slot_s[:, c:c + 1], None,
                                mybir.AluOpType.is_equal)
                            nc.tensor.matmul(
                                pT[:], xg[b][:, b7 * cpbs[b] + j, :], s_t[:],
                                start=(k == 0), stop=(k == nmm - 1))
                            k += 1
                    agg = wpool.tile([128, 128], F32, name="agg",
                                     tag="agg", bufs=3)
                    nc.vector.tensor_tensor(
                        out=agg[:], in0=pT[:],
                        in1=drep_s[:, b7 * 128:(b7 + 1) * 128],
                        op=mybir.AluOpType.mult)
                    p2 = p2pool.tile([128, cfg.d_out], F32, name="p2")
                    nc.tensor.matmul(p2[:], agg[:], w_s[:],
                                     start=True, stop=False)
                    nc.tensor.matmul(p2[:], ones_s[:], brep_s[:],
                                     start=False, stop=True)
                    nc.scalar.activation(outst[:, b7, :], p2[:],
                                         mybir.ActivationFunctionType.Relu)

                nc.sync.dma_start(
                    out=outt[sg * bpg * 128:(sg + 1) * bpg * 128, :]
                        .rearrange("(b p) d -> p b d", p=128),
                    in_=outst[:])
    nc.compile()
    return nc


_NC_CACHE = {}


def _get_nc(cfg):
    k = cfg.key()
    if k not in _NC_CACHE:
        _NC_CACHE[k] = build_nc(cfg)
    return _NC_CACHE[k]


def run(cfg, inputs, **run_kwargs):
    """Build+run on hardware; returns (full_out, BassKernelResults)."""
    in_maps, unperms = prep(cfg, inputs["x"], inputs["edge_index"],
                            inputs["weight"], inputs["bias"])
    nc = _get_nc(cfg)
    res = run_bass_kernel_spmd(nc, in_maps, list(range(cfg.n_cores)),
                               **run_kwargs)
    out = np.empty((cfg.n_nodes, cfg.d_out), np.float32)
    for m in range(cfg.n_cores):
        oc = res.results[m]["out"]
        out[m * cfg.shard:(m + 1) * cfg.shard] = oc[unperms[m]]
    return out, res


def kernel(**inputs):
    out, _ = run(FULL, inputs)
    return out


# revision 5
# speedup vs baseline: 2.4543x; 2.4543x over previous
"""GCN message-passing kernel (relu(GCNConv(x, edge_index)) w/ symmetric norm)
for 8 trn2 NeuronCores.

Math: out = relu( D^-1/2 (A+I) D^-1/2 (x @ W) + b )
Rewritten:  out[d] = relu( dinv[d] * (sum_{e: dst=d} xs[src_e]) @ W + b )
where xs[i] = dinv[i] * x[i]  (host-precomputed fp16 gather table).

Device work per core (12500 dst nodes, ~212k edges):
  - dma_gather 256B fp16 xs rows per edge (dst-sorted, binned into nbins bins
    of <=128 dst slots; src space split into 4 blocks of 25000 rows so int16
    gather indices fit; per-core block remap puts the self-loop-heavy block
    first, which gets a bigger chunk budget)
  - one-hot matmul segment sum: psum[k, slot] += xg[e,k]^T @ S[e, slot]
    (S built on DVE via iota==slot compare)
  - per bin: scale cols by dinv[dst], @W matmul (+bias via ones@brep), relu
Host: index prep / sharding / unpermute only (plus fp16 cast of x*dinv).
"""

import numpy as np

import concourse.bacc as bacc
import concourse.mybir as mybir
import concourse.tile as tile
from concourse.bass_utils import run_bass_kernel_spmd

F16 = mybir.dt.float16
F32 = mybir.dt.float32
I16 = mybir.dt.int16


class Cfg:
    def __init__(self, n_nodes, n_cores, nblk, nbins, bpg, cpbs):
        self.n_nodes = n_nodes
        self.n_cores = n_cores
        self.shard = n_nodes // n_cores      # dst nodes per core
        self.nblk = nblk                     # src blocks (int16 index range)
        self.blk = n_nodes // nblk           # rows per src block (< 32768)
        self.nbins = nbins                   # bins per core (128 dst slots each)
        self.bpg = bpg                       # bins per gather super-group
        self.nsg = nbins // bpg              # super-groups
        self.cpbs = tuple(cpbs)              # per-block chunks (128 edges) per bin
        self.caps = tuple(c * 128 for c in cpbs)
        assert len(cpbs) == nblk
        assert n_nodes % n_cores == 0 and n_nodes % nblk == 0
        assert self.blk < 32768 and nbins % bpg == 0
        assert self.shard <= nbins * 128
        # per-core block remap assumes a shard is contained in one block
        assert self.blk % self.shard == 0 or self.shard % self.blk == 0
        assert self.blk >= self.shard
        self.d_in = 128
        self.d_out = 64
        self.scpb = sum(cpbs)                # chunks per bin
        self.ncol_sg = bpg * self.scpb       # slot cols per super-group
        self.cboff = tuple(int(np.sum(cpbs[:b])) for b in range(nblk))

    def key(self):
        return (self.n_nodes, self.n_cores, self.nblk, self.nbins, self.bpg,
                self.cpbs)


FULL = Cfg(n_nodes=100000, n_cores=8, nblk=4, nbins=112, bpg=7,
           cpbs=(5, 4, 4, 4))


# ----------------------------------------------------------------------------
# host-side prep: shard / bin / build index+slot streams
# ----------------------------------------------------------------------------

def _pack_bins(cfg, degv):
    """Assign each dst (of one core's shard) to a bin s.t. per-(bin, blk) edge
    counts fit cfg.caps[blk] and bins hold <=128 dsts. Greedy fill by
    normalized worst-block load, then swap repair. Returns bin_of [shard]."""
    shard, nbins = cfg.shard, cfg.nbins
    caps = np.array(cfg.caps, np.float64)
    order = np.argsort(-degv.sum(1), kind="stable")
    loads = np.zeros((nbins, cfg.nblk), np.float64)
    counts = np.zeros(nbins, np.int64)
    bin_of = np.empty(shard, np.int64)
    for d in order:
        score = ((loads + degv[d]) / caps).max(1)
        score[counts >= 128] = np.inf
        b = int(np.argmin(score))
        bin_of[d] = b
        loads[b] += degv[d]
        counts[b] += 1

    loads = loads.astype(np.int64)
    capsi = np.array(cfg.caps, np.int64)
    for _ in range(2000):
        over = loads - capsi[None, :]
        wb, wk = np.unravel_index(np.argmax(over), over.shape)
        if over[wb, wk] <= 0:
            break
        done = False
        in_wb = np.where(bin_of == wb)[0]
        in_wb = in_wb[np.argsort(-degv[in_wb, wk])][:16]
        for tb in np.argsort(loads[:, wk])[:24]:
            if tb == wb:
                continue
            in_tb = np.where(bin_of == tb)[0]
            in_tb = in_tb[np.argsort(degv[in_tb, wk])][:16]
            for d in in_wb:
                for e in in_tb:
                    na = loads[wb] - degv[d] + degv[e]
                    nb = loads[tb] - degv[e] + degv[d]
                    if (na <= capsi).all() and (nb <= capsi).all() \
                            and na[wk] < loads[wb, wk]:
                        loads[wb], loads[tb] = na, nb
                        bin_of[d], bin_of[e] = tb, wb
                        done = True
                        break
                if done:
                    break
            if done:
                break
        if not done:
            raise RuntimeError(f"bin packing failed (load {loads.max(0)}, "
                               f"caps {cfg.caps}); raise cpbs")
    assert (loads <= capsi[None, :]).all()
    assert np.bincount(bin_of, minlength=nbins).max() <= 128
    return bin_of


def prep(cfg, x, edge_index, weight, bias):
    """Returns (in_maps, unperms). in_maps: per-core dict of named np arrays.
    unperms[m][d] = row in core m's output holding dst (m*shard + d)."""
    n, shard, nblk, blk = cfg.n_nodes, cfg.shard, cfg.nblk, cfg.blk
    nbins, bpg, nsg, cpbs, caps = cfg.nbins, cfg.bpg, cfg.nsg, cfg.cpbs, cfg.caps

    src = np.asarray(edge_index[0], dtype=np.int64)
    dst = np.asarray(edge_index[1], dtype=np.int64)
    loop = np.arange(n, dtype=np.int64)
    src_f = np.concatenate([src, loop])
    dst_f = np.concatenate([dst, loop])

    deg = np.bincount(dst_f, minlength=n).astype(np.float32)
    dinv = np.where(deg > 0, 1.0 / np.sqrt(deg), 0.0).astype(np.float32)

    xs16 = (np.asarray(x, np.float32) * dinv[:, None]).astype(np.float16)
    xb = []
    for b in range(nblk):
        t = np.zeros((blk + 1, cfg.d_in), np.float16)
        t[:blk] = xs16[b * blk:(b + 1) * blk]
        xb.append(t)

    w32 = np.asarray(weight, np.float32)                       # [128, 64]
    b32 = np.asarray(bias, np.float32)
    ones = np.ones((128, 128), np.float32)
    brep = np.tile((b32 / 128.0)[None, :], (128, 1)).astype(np.float32)

    # sort edges by dst once, globally
    order_all = np.argsort(dst_f, kind="stable")
    src_s = src_f[order_all]
    dst_s = dst_f[order_all]
    bounds = np.searchsorted(dst_s, np.arange(0, n + 1, shard))

    in_maps, unperms = [], []
    for m in range(cfg.n_cores):
        lo, hi = bounds[m], bounds[m + 1]
        e_src = src_s[lo:hi]
        e_dst = dst_s[lo:hi] - m * shard
        # per-core block remap: self-loop block (containing this shard's own
        # rows) becomes logical block 0 (which has the larger chunk budget)
        selfblk = (m * shard) // blk
        perm = [selfblk] + [b for b in range(nblk) if b != selfblk]
        inv = np.empty(nblk, np.int64)
        for lb, pb in enumerate(perm):
            inv[pb] = lb
        e_blk = inv[e_src // blk]
        degv = np.bincount(e_dst * nblk + e_blk, minlength=shard * nblk) \
                 .reshape(shard, nblk)
        bin_of = _pack_bins(cfg, degv)

        # slots: rank of dst within its bin
        counts = np.bincount(bin_of, minlength=nbins)
        starts = np.concatenate([[0], np.cumsum(counts)[:-1]])
        by_bin = np.argsort(bin_of, kind="stable")
        slot_of = np.empty(shard, np.int64)
        slot_of[by_bin] = np.arange(shard) - np.repeat(starts, counts)

        # order edges by (bin, blk, slot); position within (bin, blk) group
        b_e = bin_of[e_dst]
        s_e = slot_of[e_dst]
        es = np.lexsort((s_e, e_blk, b_e))
        gids = b_e[es] * nblk + e_blk[es]
        gcnt = np.bincount(gids, minlength=nbins * nblk)
        gstart = np.concatenate([[0], np.cumsum(gcnt)[:-1]])
        within = np.arange(len(es)) - np.repeat(gstart, gcnt)

        eb = e_blk[es]
        locs = (e_src[es] % blk).astype(np.int16)
        sl = s_e[es].astype(np.float16)
        binpos = b_e[es]

        im = {}
        slots_t = np.zeros((128, nsg * cfg.ncol_sg), np.float16)
        for b in range(nblk):
            cap = caps[b]
            call = bpg * cap
            idx_stream = np.full(nbins * cap, blk, np.int16)
            slot_stream = np.zeros(nbins * cap, np.float16)
            msk = eb == b
            p = binpos[msk] * cap + within[msk]
            idx_stream[p] = locs[msk]
            slot_stream[p] = sl[msk]

            segs = []
            for sg in range(nsg):
                seg = idx_stream[sg * call:(sg + 1) * call]
                for o in range(0, call, 1024):
                    segs.append(seg[o:o + 1024].reshape(-1, 16).T)
            arr16 = np.concatenate(segs, axis=1)
            im[f"idx{b}"] = np.tile(arr16, (8, 1)).copy()
            im[f"xb{b}"] = xb[perm[b]]

            ss = slot_stream.reshape(nbins, cpbs[b], 128)
            for sg in range(nsg):
                # col(sg, b7, b, j) = sg*ncol_sg + b7*scpb + cboff[b] + j
                cols = (sg * cfg.ncol_sg + cfg.cboff[b]
                        + np.arange(bpg)[:, None] * cfg.scpb
                        + np.arange(cpbs[b])[None, :]).ravel()
                slots_t[:, cols] = \
                    ss[sg * bpg:(sg + 1) * bpg].reshape(bpg * cpbs[b], 128).T
        im["slots"] = slots_t

        drow = np.zeros(nbins * 128, np.float32)
        drow[bin_of * 128 + slot_of] = dinv[m * shard + np.arange(shard)]
        im["drep"] = np.tile(drow[None, :], (128, 1)).copy()

        im["iotarep"] = np.tile(
            np.tile(np.arange(128, dtype=np.float16), cfg.scpb)[None, :],
            (128, 1)).copy()
        im["w"] = w32
        im["ones"] = ones
        im["brep"] = brep
        in_maps.append(im)
        unperms.append(bin_of * 128 + slot_of)
    return in_maps, unperms


# ----------------------------------------------------------------------------
# device kernel
# ----------------------------------------------------------------------------

def build_nc(cfg):
    nblk, nbins, bpg, nsg, cpbs = cfg.nblk, cfg.nbins, cfg.bpg, cfg.nsg, cfg.cpbs
    ncol_sg = cfg.ncol_sg
    nc = bacc.Bacc("TRN2", target_bir_lowering=False, debug=False,
                   num_devices=cfg.n_cores,
                   num_swdge_queues=min(nblk, 4))

    xb = [nc.dram_tensor(f"xb{b}", [cfg.blk + 1, cfg.d_in], F16,
                         kind="ExternalInput") for b in range(nblk)]
    idxt = [nc.dram_tensor(f"idx{b}", [128, nsg * bpg * cfg.caps[b] // 16], I16,
                           kind="ExternalInput") for b in range(nblk)]
    slott = nc.dram_tensor("slots", [128, nsg * ncol_sg], F16,
                           kind="ExternalInput")
    iotarept = nc.dram_tensor("iotarep", [128, cfg.scpb * 128], F16,
                              kind="ExternalInput")
    drept = nc.dram_tensor("drep", [128, nbins * 128], F32, kind="ExternalInput")
    wt = nc.dram_tensor("w", [cfg.d_in, cfg.d_out], F32, kind="ExternalInput")
    onest = nc.dram_tensor("ones", [128, 128], F32, kind="ExternalInput")
    brept = nc.dram_tensor("brep", [128, cfg.d_out], F32, kind="ExternalInput")
    outt = nc.dram_tensor("out", [nbins * 128, cfg.d_out], F32,
                          kind="ExternalOutput")

    with tile.TileContext(nc) as tc:
        with tc.tile_pool(name="const", bufs=1) as cpool, \
             tc.tile_pool(name="work", bufs=1) as wpool, \
             tc.tile_pool(name="psumT", bufs=3, space="PSUM") as ppool, \
             tc.tile_pool(name="psum2", bufs=2, space="PSUM") as p2pool:

            iotar_s = cpool.tile([128, cfg.scpb, 128], F16, name="iotar_s")
            nc.sync.dma_start(
                out=iotar_s[:],
                in_=iotarept[:].rearrange("p (c q) -> p c q", q=128))
            w_s = cpool.tile([cfg.d_in, cfg.d_out], F32, name="w_s")
            nc.sync.dma_start(out=w_s[:], in_=wt[:])
            ones_s = cpool.tile([128, 128], F32, name="ones_s")
            nc.sync.dma_start(out=ones_s[:], in_=onest[:])
            brep_s = cpool.tile([128, cfg.d_out], F32, name="brep_s")
            nc.sync.dma_start(out=brep_s[:], in_=brept[:])

            for sg in range(nsg):
                idx_tiles = []
                for b in range(nblk):
                    w16 = bpg * cfg.caps[b] // 16
                    it = wpool.tile([128, w16], I16, name=f"it{b}",
                                    tag=f"it{b}", bufs=2)
                    nc.sync.dma_start(
                        out=it[:], in_=idxt[b][:, sg * w16:(sg + 1) * w16])
                    idx_tiles.append(it)
                slot_s = wpool.tile([128, ncol_sg], F16, name="slot_s",
                                    tag="slot", bufs=2)
                nc.sync.dma_start(
                    out=slot_s[:],
                    in_=slott[:, sg * ncol_sg:(sg + 1) * ncol_sg])
                drep_s = wpool.tile([128, bpg * 128], F32, name="drep_s",
                                    tag="drep", bufs=2)
                nc.sync.dma_start(
                    out=drep_s[:],
                    in_=drept[:, sg * bpg * 128:(sg + 1) * bpg * 128])

                xg = []
                for b in range(nblk):
                    call = bpg * cfg.caps[b]
                    g = wpool.tile([128, bpg * cpbs[b], cfg.d_in], F16,
                                   name=f"xg{b}", tag=f"xg{b}", bufs=2)
                    for o in range(0, call, 1024):
                        nloc = min(1024, call - o)
                        nc.gpsimd.dma_gather(
                            g[:, o // 128:(o + nloc) // 128, :], xb[b][:],
                            idx_tiles[b][:, o // 16:(o + nloc) // 16],
                            nloc, nloc, cfg.d_in, queue_num=b % 4)
                    xg.append(g)

                outst = wpool.tile([128, bpg, cfg.d_out], F32, name="outst",
                                   tag="outst", bufs=2)
                for b7 in range(bpg):
                    pT = ppool.tile([128, 128], F32, name="pT")
                    s_big = wpool.tile([128, cfg.scpb, 128], F16,
                                       name="s_big", tag="s_big", bufs=3)
                    c0 = b7 * cfg.scpb
                    nc.vector.tensor_tensor(
                        out=s_big[:],
                        in0=slot_s[:, c0:c0 + cfg.scpb]
                            .to_broadcast([128, cfg.scpb, 128]),
                        in1=iotar_s[:],
                        op=mybir.AluOpType.is_equal)
                    k = 0
                    nmm = cfg.scpb
                    for b in range(nblk):
                        for j in range(cpbs[b]):
                            nc.tensor.matmul(
                                pT[:], xg[b][:, b7 * cpbs[b] + j, :],
                                s_big[:, cfg.cboff[b] + j, :],
                                start=(k == 0), stop=(k == nmm - 1))
                            k += 1
                    agg = wpool.tile([128, 128], F32, name="agg",
                                     tag="agg", bufs=3)
                    nc.vector.tensor_tensor(
                        out=agg[:], in0=pT[:],
                        in1=drep_s[:, b7 * 128:(b7 + 1) * 128],
                        op=mybir.AluOpType.mult)
                    p2 = p2pool.tile([128, cfg.d_out], F32, name="p2")
                    nc.tensor.matmul(p2[:], agg[:], w_s[:],
                                     start=True, stop=False)
                    nc.tensor.matmul(p2[:], ones_s[:], brep_s[:],
                                     start=False, stop=True)
                    nc.scalar.activation(outst[:, b7, :], p2[:],
                                         mybir.ActivationFunctionType.Relu)

                nc.sync.dma_start(
                    out=outt[sg * bpg * 128:(sg + 1) * bpg * 128, :]
                        .rearrange("(b p) d -> p b d", p=128),
                    in_=outst[:])
    nc.compile()
    return nc


_NC_CACHE = {}


def _get_nc(cfg):
    k = cfg.key()
    if k not in _NC_CACHE:
        _NC_CACHE[k] = build_nc(cfg)
    return _NC_CACHE[k]


def run(cfg, inputs, **run_kwargs):
    """Build+run on hardware; returns (full_out, BassKernelResults)."""
    in_maps, unperms = prep(cfg, inputs["x"], inputs["edge_index"],
                            inputs["weight"], inputs["bias"])
    nc = _get_nc(cfg)
    res = run_bass_kernel_spmd(nc, in_maps, list(range(cfg.n_cores)),
                               **run_kwargs)
    out = np.empty((cfg.n_nodes, cfg.d_out), np.float32)
    for m in range(cfg.n_cores):
        oc = res.results[m]["out"]
        out[m * cfg.shard:(m + 1) * cfg.shard] = oc[unperms[m]]
    return out, res


def kernel(**inputs):
    out, _ = run(FULL, inputs)
    return out


# revision 6
# speedup vs baseline: 3.2848x; 1.3384x over previous
"""GCN message-passing kernel (relu(GCNConv(x, edge_index)) w/ symmetric norm)
for 8 trn2 NeuronCores.

Math: out = relu( D^-1/2 (A+I) D^-1/2 (x @ W) + b )
Rewritten:  out[d] = relu( dinv[d] * (sum_{e: dst=d} xs[src_e]) @ W + b )
where xs[i] = dinv[i] * x[i]  (host-precomputed fp16 gather table).

Device work per core (12500 dst nodes, ~212k edges):
  - dma_gather 256B fp16 xs rows per edge (dst-sorted, binned into nbins bins
    of <=128 dst slots; src space split into 4 blocks of 25000 rows so int16
    gather indices fit; per-core block remap puts the self-loop-heavy block
    first, which gets a bigger chunk budget)
  - one-hot matmul segment sum: psum[k, slot] += xg[e,k]^T @ S[e, slot]
    (S built on DVE via iota==slot compare)
  - per bin: scale cols by dinv[dst], @W matmul (+bias via ones@brep), relu
Host: index prep / sharding / unpermute only (plus fp16 cast of x*dinv).
"""

import numpy as np

import concourse.bacc as bacc
import concourse.mybir as mybir
import concourse.tile as tile
from concourse.bass_utils import run_bass_kernel_spmd

F16 = mybir.dt.float16
F32 = mybir.dt.float32
I16 = mybir.dt.int16


class Cfg:
    def __init__(self, n_nodes, n_cores, nblk, nbins, bpg, cpbs):
        self.n_nodes = n_nodes
        self.n_cores = n_cores
        self.shard = n_nodes // n_cores      # dst nodes per core
        self.nblk = nblk                     # src blocks (int16 index range)
        self.blk = n_nodes // nblk           # rows per src block (< 32768)
        self.nbins = nbins                   # bins per core (128 dst slots each)
        self.bpg = bpg                       # bins per gather super-group
        self.nsg = nbins // bpg              # super-groups
        self.cpbs = tuple(cpbs)              # per-block chunks (128 edges) per bin
        self.caps = tuple(c * 128 for c in cpbs)
        assert len(cpbs) == nblk
        assert n_nodes % n_cores == 0 and n_nodes % nblk == 0
        assert self.blk < 32768 and nbins % bpg == 0
        assert self.shard <= nbins * 128
        # per-core block remap assumes a shard is contained in one block
        assert self.blk % self.shard == 0 or self.shard % self.blk == 0
        assert self.blk >= self.shard
        self.d_in = 128
        self.d_out = 64
        self.scpb = sum(cpbs)                # chunks per bin
        self.ncol_sg = bpg * self.scpb       # slot cols per super-group
        self.cboff = tuple(int(np.sum(cpbs[:b])) for b in range(nblk))

    def key(self):
        return (self.n_nodes, self.n_cores, self.nblk, self.nbins, self.bpg,
                self.cpbs)


FULL = Cfg(n_nodes=100000, n_cores=8, nblk=4, nbins=112, bpg=7,
           cpbs=(5, 4, 4, 4))


# ----------------------------------------------------------------------------
# host-side prep: shard / bin / build index+slot streams
# ----------------------------------------------------------------------------

def _pack_bins(cfg, degv):
    """Assign each dst (of one core's shard) to a bin s.t. per-(bin, blk) edge
    counts fit cfg.caps[blk] and bins hold <=128 dsts. Greedy fill by
    normalized worst-block load, then swap repair. Returns bin_of [shard]."""
    shard, nbins = cfg.shard, cfg.nbins
    caps = np.array(cfg.caps, np.float64)
    order = np.argsort(-degv.sum(1), kind="stable")
    loads = np.zeros((nbins, cfg.nblk), np.float64)
    counts = np.zeros(nbins, np.int64)
    bin_of = np.empty(shard, np.int64)
    for d in order:
        score = ((loads + degv[d]) / caps).max(1)
        score[counts >= 128] = np.inf
        b = int(np.argmin(score))
        bin_of[d] = b
        loads[b] += degv[d]
        counts[b] += 1

    loads = loads.astype(np.int64)
    capsi = np.array(cfg.caps, np.int64)
    for _ in range(2000):
        over = loads - capsi[None, :]
        wb, wk = np.unravel_index(np.argmax(over), over.shape)
        if over[wb, wk] <= 0:
            break
        done = False
        in_wb = np.where(bin_of == wb)[0]
        in_wb = in_wb[np.argsort(-degv[in_wb, wk])][:16]
        for tb in np.argsort(loads[:, wk])[:24]:
            if tb == wb:
                continue
            in_tb = np.where(bin_of == tb)[0]
            in_tb = in_tb[np.argsort(degv[in_tb, wk])][:16]
            for d in in_wb:
                for e in in_tb:
                    na = loads[wb] - degv[d] + degv[e]
                    nb = loads[tb] - degv[e] + degv[d]
                    if (na <= capsi).all() and (nb <= capsi).all() \
                            and na[wk] < loads[wb, wk]:
                        loads[wb], loads[tb] = na, nb
                        bin_of[d], bin_of[e] = tb, wb
                        done = True
                        break
                if done:
                    break
            if done:
                break
        if not done:
            raise RuntimeError(f"bin packing failed (load {loads.max(0)}, "
                               f"caps {cfg.caps}); raise cpbs")
    assert (loads <= capsi[None, :]).all()
    assert np.bincount(bin_of, minlength=nbins).max() <= 128
    return bin_of


def prep(cfg, x, edge_index, weight, bias):
    """Returns (in_maps, unperms). in_maps: per-core dict of named np arrays.
    unperms[m][d] = row in core m's output holding dst (m*shard + d)."""
    n, shard, nblk, blk = cfg.n_nodes, cfg.shard, cfg.nblk, cfg.blk
    nbins, bpg, nsg, cpbs, caps = cfg.nbins, cfg.bpg, cfg.nsg, cfg.cpbs, cfg.caps

    src = np.asarray(edge_index[0], dtype=np.int64)
    dst = np.asarray(edge_index[1], dtype=np.int64)
    loop = np.arange(n, dtype=np.int64)
    src_f = np.concatenate([src, loop])
    dst_f = np.concatenate([dst, loop])

    deg = np.bincount(dst_f, minlength=n).astype(np.float32)
    dinv = np.where(deg > 0, 1.0 / np.sqrt(deg), 0.0).astype(np.float32)

    xs16 = (np.asarray(x, np.float32) * dinv[:, None]).astype(np.float16)
    xb = []
    for b in range(nblk):
        t = np.zeros((blk + 1, cfg.d_in), np.float16)
        t[:blk] = xs16[b * blk:(b + 1) * blk]
        xb.append(t)

    w32 = np.asarray(weight, np.float32)                       # [128, 64]
    b32 = np.asarray(bias, np.float32)
    ones = np.ones((128, 128), np.float32)
    brep = np.tile((b32 / 128.0)[None, :], (128, 1)).astype(np.float32)

    # sort edges by dst once, globally
    order_all = np.argsort(dst_f, kind="stable")
    src_s = src_f[order_all]
    dst_s = dst_f[order_all]
    bounds = np.searchsorted(dst_s, np.arange(0, n + 1, shard))

    in_maps, unperms = [], []
    for m in range(cfg.n_cores):
        lo, hi = bounds[m], bounds[m + 1]
        e_src = src_s[lo:hi]
        e_dst = dst_s[lo:hi] - m * shard
        # per-core block remap: self-loop block (containing this shard's own
        # rows) becomes logical block 0 (which has the larger chunk budget)
        selfblk = (m * shard) // blk
        perm = [selfblk] + [b for b in range(nblk) if b != selfblk]
        inv = np.empty(nblk, np.int64)
        for lb, pb in enumerate(perm):
            inv[pb] = lb
        e_blk = inv[e_src // blk]
        degv = np.bincount(e_dst * nblk + e_blk, minlength=shard * nblk) \
                 .reshape(shard, nblk)
        bin_of = _pack_bins(cfg, degv)

        # slots: rank of dst within its bin
        counts = np.bincount(bin_of, minlength=nbins)
        starts = np.concatenate([[0], np.cumsum(counts)[:-1]])
        by_bin = np.argsort(bin_of, kind="stable")
        slot_of = np.empty(shard, np.int64)
        slot_of[by_bin] = np.arange(shard) - np.repeat(starts, counts)

        # order edges by (bin, blk, slot); position within (bin, blk) group
        b_e = bin_of[e_dst]
        s_e = slot_of[e_dst]
        es = np.lexsort((s_e, e_blk, b_e))
        gids = b_e[es] * nblk + e_blk[es]
        gcnt = np.bincount(gids, minlength=nbins * nblk)
        gstart = np.concatenate([[0], np.cumsum(gcnt)[:-1]])
        within = np.arange(len(es)) - np.repeat(gstart, gcnt)

        eb = e_blk[es]
        locs = (e_src[es] % blk).astype(np.int16)
        sl = s_e[es].astype(np.float16)
        binpos = b_e[es]

        im = {}
        slots_t = np.zeros((128, nsg * cfg.ncol_sg), np.float16)
        for b in range(nblk):
            cap = caps[b]
            call = bpg * cap
            idx_stream = np.full(nbins * cap, blk, np.int16)
            slot_stream = np.zeros(nbins * cap, np.float16)
            msk = eb == b
            p = binpos[msk] * cap + within[msk]
            idx_stream[p] = locs[msk]
            slot_stream[p] = sl[msk]

            segs = []
            for sg in range(nsg):
                seg = idx_stream[sg * call:(sg + 1) * call]
                for o in range(0, call, 1024):
                    segs.append(seg[o:o + 1024].reshape(-1, 16).T)
            arr16 = np.concatenate(segs, axis=1)
            im[f"idx{b}"] = np.tile(arr16, (8, 1)).copy()
            im[f"xb{b}"] = xb[perm[b]]

            ss = slot_stream.reshape(nbins, cpbs[b], 128)
            for sg in range(nsg):
                # col(sg, b7, b, j) = sg*ncol_sg + b7*scpb + cboff[b] + j
                cols = (sg * cfg.ncol_sg + cfg.cboff[b]
                        + np.arange(bpg)[:, None] * cfg.scpb
                        + np.arange(cpbs[b])[None, :]).ravel()
                slots_t[:, cols] = \
                    ss[sg * bpg:(sg + 1) * bpg].reshape(bpg * cpbs[b], 128).T
        im["slots"] = slots_t

        drow = np.zeros(nbins * 128, np.float32)
        drow[bin_of * 128 + slot_of] = dinv[m * shard + np.arange(shard)]
        im["drep"] = np.tile(drow[None, :], (128, 1)).copy()

        im["iotarep"] = np.tile(
            np.tile(np.arange(128, dtype=np.float16), cfg.scpb)[None, :],
            (128, 1)).copy()
        im["w"] = w32
        im["ones"] = ones
        im["brep"] = brep
        in_maps.append(im)
        unperms.append(bin_of * 128 + slot_of)
    return in_maps, unperms


# ----------------------------------------------------------------------------
# device kernel
# ----------------------------------------------------------------------------

def build_nc(cfg):
    nblk, nbins, bpg, nsg, cpbs = cfg.nblk, cfg.nbins, cfg.bpg, cfg.nsg, cfg.cpbs
    ncol_sg = cfg.ncol_sg
    nc = bacc.Bacc("TRN2", target_bir_lowering=False, debug=False,
                   num_devices=cfg.n_cores,
                   num_swdge_queues=min(nblk, 4))

    xb = [nc.dram_tensor(f"xb{b}", [cfg.blk + 1, cfg.d_in], F16,
                         kind="ExternalInput") for b in range(nblk)]
    idxt = [nc.dram_tensor(f"idx{b}", [128, nsg * bpg * cfg.caps[b] // 16], I16,
                           kind="ExternalInput") for b in range(nblk)]
    slott = nc.dram_tensor("slots", [128, nsg * ncol_sg], F16,
                           kind="ExternalInput")
    iotarept = nc.dram_tensor("iotarep", [128, cfg.scpb * 128], F16,
                              kind="ExternalInput")
    drept = nc.dram_tensor("drep", [128, nbins * 128], F32, kind="ExternalInput")
    wt = nc.dram_tensor("w", [cfg.d_in, cfg.d_out], F32, kind="ExternalInput")
    onest = nc.dram_tensor("ones", [128, 128], F32, kind="ExternalInput")
    brept = nc.dram_tensor("brep", [128, cfg.d_out], F32, kind="ExternalInput")
    outt = nc.dram_tensor("out", [nbins * 128, cfg.d_out], F32,
                          kind="ExternalOutput")

    with tile.TileContext(nc) as tc:
        with tc.tile_pool(name="const", bufs=1) as cpool, \
             tc.tile_pool(name="work", bufs=1) as wpool, \
             tc.tile_pool(name="psumT", bufs=3, space="PSUM") as ppool, \
             tc.tile_pool(name="psum2", bufs=2, space="PSUM") as p2pool:

            iotar_s = cpool.tile([128, cfg.scpb, 128], F16, name="iotar_s")
            nc.sync.dma_start(
                out=iotar_s[:],
                in_=iotarept[:].rearrange("p (c q) -> p c q", q=128))
            w_s = cpool.tile([cfg.d_in, cfg.d_out], F32, name="w_s")
            nc.sync.dma_start(out=w_s[:], in_=wt[:])
            ones_s = cpool.tile([128, 128], F32, name="ones_s")
            nc.sync.dma_start(out=ones_s[:], in_=onest[:])
            brep_s = cpool.tile([128, cfg.d_out], F32, name="brep_s")
            nc.sync.dma_start(out=brep_s[:], in_=brept[:])

            for sg in range(nsg):
                idx_tiles = []
                for b in range(nblk):
                    w16 = bpg * cfg.caps[b] // 16
                    it = wpool.tile([128, w16], I16, name=f"it{b}",
                                    tag=f"it{b}", bufs=3)
                    nc.sync.dma_start(
                        out=it[:], in_=idxt[b][:, sg * w16:(sg + 1) * w16])
                    idx_tiles.append(it)
                slot_s = wpool.tile([128, ncol_sg], F16, name="slot_s",
                                    tag="slot", bufs=2)
                nc.sync.dma_start(
                    out=slot_s[:],
                    in_=slott[:, sg * ncol_sg:(sg + 1) * ncol_sg])
                drep_s = wpool.tile([128, bpg * 128], F32, name="drep_s",
                                    tag="drep", bufs=2)
                nc.sync.dma_start(
                    out=drep_s[:],
                    in_=drept[:, sg * bpg * 128:(sg + 1) * bpg * 128])

                xg = []
                subcalls = []
                for b in range(nblk):
                    call = bpg * cfg.caps[b]
                    g = wpool.tile([128, bpg * cpbs[b], cfg.d_in], F16,
                                   name=f"xg{b}", tag=f"xg{b}", bufs=3)
                    xg.append(g)
                    for o in range(0, call, 1024):
                        subcalls.append((b, o, min(1024, call - o)))
                # round-robin across blocks so the 4 SWDGE queues fill evenly
                subcalls.sort(key=lambda t: (t[1], t[0]))
                for b, o, nloc in subcalls:
                    nc.gpsimd.dma_gather(
                        xg[b][:, o // 128:(o + nloc) // 128, :], xb[b][:],
                        idx_tiles[b][:, o // 16:(o + nloc) // 16],
                        nloc, nloc, cfg.d_in, queue_num=b % 4)

                outst = wpool.tile([128, bpg, cfg.d_out], F32, name="outst",
                                   tag="outst", bufs=2)
                for b7 in range(bpg):
                    pT = ppool.tile([128, 128], F32, name="pT")
                    s_big = wpool.tile([128, cfg.scpb, 128], F16,
                                       name="s_big", tag="s_big", bufs=3)
                    c0 = b7 * cfg.scpb
                    nc.vector.tensor_tensor(
                        out=s_big[:],
                        in0=slot_s[:, c0:c0 + cfg.scpb]
                            .to_broadcast([128, cfg.scpb, 128]),
                        in1=iotar_s[:],
                        op=mybir.AluOpType.is_equal)
                    k = 0
                    nmm = cfg.scpb
                    for b in range(nblk):
                        for j in range(cpbs[b]):
                            nc.tensor.matmul(
                                pT[:], xg[b][:, b7 * cpbs[b] + j, :],
                                s_big[:, cfg.cboff[b] + j, :],
                                start=(k == 0), stop=(k == nmm - 1))
                            k += 1
                    agg = wpool.tile([128, 128], F32, name="agg",
                                     tag="agg", bufs=3)
                    nc.vector.tensor_tensor(
                        out=agg[:], in0=pT[:],
                        in1=drep_s[:, b7 * 128:(b7 + 1) * 128],
                        op=mybir.AluOpType.mult)
                    p2 = p2pool.tile([128, cfg.d_out], F32, name="p2")
                    nc.tensor.matmul(p2[:], agg[:], w_s[:],
                                     start=True, stop=False)
                    nc.tensor.matmul(p2[:], ones_s[:], brep_s[:],
                                     start=False, stop=True)
                    nc.scalar.activation(outst[:, b7, :], p2[:],
                                         mybir.ActivationFunctionType.Relu)

                nc.sync.dma_start(
                    out=outt[sg * bpg * 128:(sg + 1) * bpg * 128, :]
                        .rearrange("(b p) d -> p b d", p=128),
                    in_=outst[:])
    nc.compile()
    return nc


_NC_CACHE = {}


def _get_nc(cfg):
    k = cfg.key()
    if k not in _NC_CACHE:
        _NC_CACHE[k] = build_nc(cfg)
    return _NC_CACHE[k]


def run(cfg, inputs, **run_kwargs):
    """Build+run on hardware; returns (full_out, BassKernelResults)."""
    in_maps, unperms = prep(cfg, inputs["x"], inputs["edge_index"],
                            inputs["weight"], inputs["bias"])
    nc = _get_nc(cfg)
    res = run_bass_kernel_spmd(nc, in_maps, list(range(cfg.n_cores)),
                               **run_kwargs)
    out = np.empty((cfg.n_nodes, cfg.d_out), np.float32)
    for m in range(cfg.n_cores):
        oc = res.results[m]["out"]
        out[m * cfg.shard:(m + 1) * cfg.shard] = oc[unperms[m]]
    return out, res


def kernel(**inputs):
    out, _ = run(FULL, inputs)
    return out
